# revision 1
# baseline (speedup 1.0000x reference)
"""Trainium2 Bass kernel: CapsuleLayer with dynamic routing (3 iterations).

Full inputs in, full output out. Internally: data-parallel over batch,
B=32 -> 4 examples per core across 8 NeuronCores.

Per-core layout: positions p = (b_local, h*w) = 4*144 = 576 (+64 pad = 5
chunks of 128 partitions). Features live on the free dim.

  votes[p, i, a, d]  (bf16, d innermost)  <- PE matmuls, x as stationary
  routing (softmax over d / weighted sums / squash) -> DVE + ACT ops

Iteration 0 is algebraically collapsed onto the PE: route is uniform
(=1/OD), so preact0 = 0.1*sum_i votes + bias is computed directly from x
with a 0.1-scaled weight (K=512 accumulated matmul) plus a ones-row bias
matmul.
"""

import os
import sys

if "/opt/trn_rl_repo" not in sys.path:
    sys.path.insert(0, "/opt/trn_rl_repo")
# recover automatically if a previous run left a NeuronCore wedged
os.environ.setdefault("NEURON_RT_RESET_CORES", "1")

import numpy as np

import concourse.bacc as bacc
import concourse.bass as bass
import concourse.mybir as mybir
import concourse.tile as tile

# problem constants (hardcoded per contract)
B, I, A, OD, OA, H, W = 32, 32, 16, 10, 16, 12, 12
HW = H * W              # 144
NCORES = 8
BL = B // NCORES        # 4 examples per core
NPOS = BL * HW          # 576 real positions per core
NCH = 5                 # chunks of 128 positions (640 incl. pad)
PADPOS = NCH * 128      # 640
KF = OD * OA            # 160 output features per (pos, i)
VF = I * KF             # 5120 votes features per position
NT = 8                  # x/w interleaved tiles (4 i's each, 32-aligned)

f32 = mybir.dt.float32
bf16 = mybir.dt.bfloat16
ALU = mybir.AluOpType
AX = mybir.AxisListType
ACTF = mybir.ActivationFunctionType


def _v_iad(ap_2d, c):
    """votes chunk view [p, i, a, d]."""
    return ap_2d[:, c * VF:(c + 1) * VF].rearrange(
        "p (i a d) -> p i a d", i=I, a=OA, d=OD)


def emit_kernel(tc, x_d, w_d, bias_d, out_d, stage=99, rep=1):
    nc = tc.nc

    with (
        tc.tile_pool(name="const", bufs=1) as cpool,
        tc.tile_pool(name="work", bufs=4) as wpool,
        tc.tile_pool(name="small", bufs=4) as spool,
        tc.tile_pool(name="vps", bufs=5, space="PSUM") as vps,
        tc.tile_pool(name="s0ps", bufs=2, space="PSUM") as s0ps,
    ):
        # ---------------- persistent tiles ----------------
        votes2 = cpool.tile([128, NCH * VF], bf16, name="votes2", tag="votes2")
        x_int = [cpool.tile([128, PADPOS], bf16, name=f"x_int{t}", tag=f"x_int{t}") for t in range(NT)]
        w_int = [cpool.tile([128, KF], bf16, name=f"w_int{t}", tag=f"w_int{t}") for t in range(NT)]
        wsc_int = [cpool.tile([128, KF], bf16, name=f"wsc{t}", tag=f"wsc{t}") for t in range(NT)]
        bias_b = cpool.tile([1, KF], bf16, name="bias_b", tag="bias_b")
        ones_sb = cpool.tile([1, 128], bf16, name="ones_sb", tag="ones_sb")
        bias_bc = cpool.tile([128, KF], f32, name="bias_bc", tag="bias_bc")
        logits = cpool.tile([128, NCH * I * OD], bf16, name="logits", tag="logits")
        route_b = cpool.tile([128, NCH * I * OD], bf16, name="route_b", tag="route_b")
        act_b = cpool.tile([128, NCH * KF], bf16, name="act_b", tag="act_b")
        act2 = cpool.tile([128, NCH * KF], f32, name="act2", tag="act2")

        # ---------------- staging + setup (pool scoped to setup so its
        # 20KB/partition is reusable by the work pool afterwards) --------
        setup_stack = tc.tile_pool(name="stage", bufs=1)
        stpool = setup_stack.__enter__()
        x_stage = [stpool.tile([128, PADPOS], f32, name=f"x_st{t}", tag=f"x_st{t}") for t in range(NT)]
        w_stage = [stpool.tile([128, KF], f32, name=f"w_st{t}", tag=f"w_st{t}") for t in range(NT)]
        bias_st = stpool.tile([1, KF], f32, name="bias_st", tag="bias_st")

        for t in range(NT):
            nc.gpsimd.memset(x_stage[t][:], 0.0)
            nc.gpsimd.memset(w_stage[t][:], 0.0)
        nc.gpsimd.memset(ones_sb[:], 1.0)

        # i -> (tile t = i%8, row-group g = i//8). Same-bank votes matmuls
        # (consecutive i) then share a row-group (PE serializes them), while
        # the 4 concurrent row-groups write 4 different PSUM banks --
        # concurrent writes into one PSUM bank crash the device.
        # DMA APs are limited to 3 dims, so one DMA per i. Order tile-major
        # with the 4 row-groups of each tile split across both HWDGE engines
        # (SP + ACT) so each tile is fully landed (and convertible) early.
        for t in range(NT):
            for g in range(4):
                i = 8 * g + t
                eng = nc.sync if g % 2 == 0 else nc.scalar
                eng.dma_start(
                    x_stage[t][g * 32:g * 32 + A, 0:NPOS].rearrange(
                        "p (b hw) -> p b hw", b=BL),
                    x_d[:, i].rearrange("b a h w -> a b (h w)"),
                )
                eng.dma_start(
                    w_stage[t][g * 32:g * 32 + A, :],
                    w_d[i],
                )
            nc.vector.tensor_copy(x_int[t][:], x_stage[t][:])
            nc.vector.tensor_copy(w_int[t][:], w_stage[t][:])
            nc.vector.tensor_scalar_mul(wsc_int[t][:], w_int[t][:], 1.0 / OD)
        # bias[d, a] -> [1, (a d)]
        nc.sync.dma_start(
            bias_st[0:1, :].rearrange("p (a d) -> p a d", a=OA),
            bias_d.rearrange("d a -> a d").unsqueeze(0),
        )
        nc.vector.tensor_copy(bias_b[:], bias_st[:])
        setup_stack.__exit__(None, None, None)

        # bias broadcast to all partitions via ones-row matmul
        bps = s0ps.tile([128, KF], f32, name="biasps", tag="biasps", bufs=1)
        nc.tensor.matmul(bps[:], lhsT=ones_sb[:, 0:128], rhs=bias_b[:],
                         start=True, stop=True, tile_position=(0, 0))
        nc.scalar.copy(bias_bc[:], bps[:])

        # (a, d) view of a w tile slice: w rows are stored k = d*OA + a
        def w_ad(wt, g):
            return wt[g * 32:g * 32 + A, :].rearrange("p (d a) -> p a d", d=OD)

        # ---------------- helpers ----------------
        def emit_squash(pre_ap, dst_ad_ap):
            """dst = pre * |pre|_a/(1+|pre|_a^2), norm over atom dim a.

            pre_ap/dst views are [p, a, d]."""
            sq = spool.tile([128, KF], f32, name="sq", tag="sq")
            nc.vector.tensor_mul(sq[:], pre_ap, pre_ap)
            ss = spool.tile([128, OD], f32, name="ss", tag="ss")
            nc.vector.tensor_reduce(
                out=ss[:], in_=sq[:].rearrange("p (a d) -> p d a", a=OA),
                axis=AX.X, op=ALU.add)
            # sqrt via exp(0.5*ln(ss)): keeps ACT on one table set (exp+ln)
            # instead of thrashing exp<->sqrt set loads every iteration.
            lt = spool.tile([128, OD], f32, name="lt", tag="lt")
            nc.scalar.activation(lt[:], ss[:], ACTF.Ln)
            st = spool.tile([128, OD], f32, name="st", tag="st")
            nc.scalar.activation(st[:], lt[:], ACTF.Exp, scale=0.5)
            dn = spool.tile([128, OD], f32, name="dn", tag="dn")
            nc.vector.tensor_scalar_add(dn[:], ss[:], 1.0)
            di = spool.tile([128, OD], f32, name="di", tag="di")
            nc.vector.reciprocal(di[:], dn[:])
            sc = spool.tile([128, OD], f32, name="sc", tag="sc")
            nc.vector.tensor_mul(sc[:], st[:], di[:])
            nc.vector.tensor_tensor(
                out=dst_ad_ap,
                in0=pre_ap.rearrange("p (a d) -> p a d", a=OA),
                in1=sc[:].unsqueeze(1).broadcast_to([128, OA, OD]),
                op=ALU.mult)

        def emit_agree(c, dst_id_ap):
            """dst[p, i, d] (f32) = sum_a votes[p,i,a,d] * act[p,a,d]."""
            vv = _v_iad(votes2[:], c)
            ab = act_b[:, c * KF:(c + 1) * KF].rearrange(
                "p (a d) -> p a d", a=OA).unsqueeze(1).broadcast_to(
                [128, I, OA, OD])
            y = wpool.tile([128, VF], bf16, name="y", tag="y")
            yv = y[:].rearrange("p (i a d) -> p i a d", i=I, a=OA)
            # parallel DVE/GPSIMD split at the engines' ~4.2:1 rate ratio
            nc.vector.tensor_mul(yv[:, 0:26], vv[:, 0:26], ab[:, 0:26])
            nc.gpsimd.tensor_mul(yv[:, 26:I], vv[:, 26:I], ab[:, 26:I])
            t1 = wpool.tile([128, I * 8 * OD], bf16, name="t1", tag="t1")
            t1v = t1[:].rearrange("p (i a d) -> p i a d", i=I, a=8)
            nc.vector.tensor_add(t1v[:, 0:26], yv[:, 0:26, 0:8, :],
                                 yv[:, 0:26, 8:16, :])
            nc.gpsimd.tensor_add(t1v[:, 26:I], yv[:, 26:I, 0:8, :],
                                 yv[:, 26:I, 8:16, :])
            t2 = wpool.tile([128, I * 4 * OD], bf16, name="t2", tag="t2")
            t2v = t2[:].rearrange("p (i a d) -> p i a d", i=I, a=4)
            nc.vector.tensor_add(t2v[:, 0:26], t1v[:, 0:26, 0:4, :],
                                 t1v[:, 0:26, 4:8, :])
            nc.gpsimd.tensor_add(t2v[:, 26:I], t1v[:, 26:I, 0:4, :],
                                 t1v[:, 26:I, 4:8, :])
            t3 = wpool.tile([128, I * 2 * OD], bf16, name="t3", tag="t3")
            t3v = t3[:].rearrange("p (i a d) -> p i a d", i=I, a=2)
            nc.vector.tensor_add(t3v[:, 0:26], t2v[:, 0:26, 0:2, :],
                                 t2v[:, 0:26, 2:4, :])
            nc.gpsimd.tensor_add(t3v[:, 26:I], t2v[:, 26:I, 0:2, :],
                                 t2v[:, 26:I, 2:4, :])
            nc.vector.tensor_add(
                dst_id_ap, t3v[:, :, 0, :], t3v[:, :, 1, :])

        def emit_softmax(c):
            """route_b[c] = softmax over d of logits[c]."""
            lg = logits[:, c * I * OD:(c + 1) * I * OD]
            eu = spool.tile([128, I * OD], bf16, name="eu", tag="eu")
            nc.scalar.activation(eu[:], lg, ACTF.Exp)
            z = spool.tile([128, I], f32, name="z", tag="z")
            nc.vector.tensor_reduce(
                out=z[:], in_=eu[:].rearrange("p (i d) -> p i d", i=I),
                axis=AX.X, op=ALU.add)
            zr = spool.tile([128, I], f32, name="zr", tag="zr")
            nc.vector.reciprocal(zr[:], z[:])
            rb = route_b[:, c * I * OD:(c + 1) * I * OD]
            nc.vector.tensor_tensor(
                out=rb.rearrange("p (i d) -> p i d", i=I),
                in0=eu[:].rearrange("p (i d) -> p i d", i=I),
                in1=zr[:].unsqueeze(2).broadcast_to([128, I, OD]),
                op=ALU.mult)

        def emit_preact(c):
            """pre[p, (a d)] f32 = sum_i route[p,i,d]*votes[p,i,a,d] + bias."""
            vv = _v_iad(votes2[:], c)
            rb = route_b[:, c * I * OD:(c + 1) * I * OD].rearrange(
                "p (i d) -> p i d", i=I).unsqueeze(2).broadcast_to(
                [128, I, OA, OD])
            y = wpool.tile([128, VF], bf16, name="y", tag="y")
            y4 = y[:].rearrange("p (i a d) -> p i a d", i=I, a=OA)
            nc.vector.tensor_mul(y4[:, 0:26], vv[:, 0:26], rb[:, 0:26])
            nc.gpsimd.tensor_mul(y4[:, 26:I], vv[:, 26:I], rb[:, 26:I])
            u1 = wpool.tile([128, VF // 2], bf16, name="t1", tag="t1")
            nc.vector.tensor_add(u1[:, 0:2080], y[:, 0:2080],
                                 y[:, VF // 2:VF // 2 + 2080])
            nc.gpsimd.tensor_add(u1[:, 2080:VF // 2], y[:, 2080:VF // 2],
                                 y[:, VF // 2 + 2080:VF])
            u2 = wpool.tile([128, VF // 4], bf16, name="t2", tag="t2")
            nc.vector.tensor_add(u2[:, 0:1040], u1[:, 0:1040],
                                 u1[:, VF // 4:VF // 4 + 1040])
            nc.gpsimd.tensor_add(u2[:, 1040:VF // 4], u1[:, 1040:VF // 4],
                                 u1[:, VF // 4 + 1040:VF // 2])
            u3 = wpool.tile([128, VF // 8], bf16, name="t3", tag="t3")
            nc.vector.tensor_add(u3[:, 0:520], u2[:, 0:520],
                                 u2[:, VF // 8:VF // 8 + 520])
            nc.gpsimd.tensor_add(u3[:, 520:VF // 8], u2[:, 520:VF // 8],
                                 u2[:, VF // 8 + 520:VF // 4])
            u4 = wpool.tile([128, VF // 16], bf16, name="t4", tag="t4")
            nc.vector.tensor_add(u4[:], u3[:, 0:VF // 16], u3[:, VF // 16:VF // 8])
            pre = spool.tile([128, KF], f32, name="pre", tag="pre")
            nc.vector.tensor_add(pre[:], u4[:, 0:KF], u4[:, KF:2 * KF])
            nc.vector.tensor_add(pre[:], pre[:], bias_bc[:])
            return pre

        if stage < 2:
            # stage 1: setup only; dump x_int as output via act2 dma
            for c in range(NCH):
                nc.vector.tensor_copy(act2[:, c * KF:(c + 1) * KF],
                                      x_int[0][:, c * 96:c * 96 + KF])
                nc.sync.dma_start(out_d[c], act2[:, c * KF:(c + 1) * KF])
            return

        def emit_votes(c, first=False):
            # votes generation on PE, drained to SBUF bf16 by ACT.
            # One PSUM tile (bank) per triple of same-row-group i's: the PE
            # serializes same-row-group matmuls, and the 4 row groups (one
            # per g) run concurrently into 4 different banks.
            for trip in range(3):
                for g in range(4):
                    js = list(range(3 * trip, min(3 * trip + 3, 8)))
                    n = len(js)
                    i0 = 8 * g + js[0]
                    vt = vps.tile([128, 3 * KF], f32, name="vt", tag="vt")
                    for idx, j in enumerate(js):
                        nc.tensor.matmul(
                            vt[:, idx * KF:(idx + 1) * KF],
                            lhsT=x_int[j][g * 32:g * 32 + A,
                                          c * 128:(c + 1) * 128],
                            rhs=w_ad(w_int[j], g),
                            start=True, stop=True,
                            tile_position=(g * 32, 0))
                    if first and g % 2 == 0:
                        # chunk-0 startup: DVE is idle; splitting drains
                        # across DVE+ACT halves the drain tail latency
                        nc.vector.tensor_copy(
                            votes2[:, c * VF + i0 * KF: c * VF + (i0 + n) * KF],
                            vt[:, 0:n * KF])
                    else:
                        nc.scalar.copy(
                            votes2[:, c * VF + i0 * KF: c * VF + (i0 + n) * KF],
                            vt[:, 0:n * KF])

        # ---------------- main chunk loop (software-pipelined emission:
        # chunk c+1's votes-gen emitted before chunk c's routing so its
        # PE matmuls + ACT drains outrank the routing small-ops) --------
        emit_votes(0, first=True)
        for cc in range(NCH * rep):
            c = cc % NCH
            if cc + 1 < NCH * rep:
                emit_votes((cc + 1) % NCH)

            if stage < 3:
                nc.vector.tensor_copy(
                    act2[:, c * KF:(c + 1) * KF],
                    votes2[:, c * VF:c * VF + KF])
                nc.sync.dma_start(out_d[c], act2[:, c * KF:(c + 1) * KF])
                continue

            # iter 0 collapsed: preact0 = 0.1*sum_i votes + bias (on PE)
            s0t = s0ps.tile([128, KF], f32, name="s0", tag="s0")
            for t in range(NT):
                nc.tensor.matmul(
                    s0t[:],
                    lhsT=x_int[t][:, c * 128:(c + 1) * 128],
                    rhs=wsc_int[t][:].rearrange("p (d a) -> p a d", d=OD),
                    start=(t == 0), stop=False, tile_position=(0, 0))
            nc.tensor.matmul(s0t[:], lhsT=ones_sb[:, 0:128], rhs=bias_b[:],
                             start=False, stop=True, tile_position=(0, 0))
            pre0 = spool.tile([128, KF], f32, name="pre", tag="pre")
            nc.scalar.copy(pre0[:], s0t[:])
            emit_squash(pre0[:], act_b[:, c * KF:(c + 1) * KF].rearrange(
                "p (a d) -> p a d", a=OA))

            if stage < 4:
                nc.vector.tensor_copy(
                    act2[:, c * KF:(c + 1) * KF],
                    act_b[:, c * KF:(c + 1) * KF])
                nc.sync.dma_start(out_d[c], act2[:, c * KF:(c + 1) * KF])
                continue

            # iter 0 agreement -> logits
            emit_agree(c, logits[:, c * I * OD:(c + 1) * I * OD].rearrange(
                "p (i d) -> p i d", i=I))

            if stage < 5:
                nc.vector.tensor_copy(
                    act2[:, c * KF:(c + 1) * KF],
                    logits[:, c * I * OD:c * I * OD + KF])
                nc.sync.dma_start(out_d[c], act2[:, c * KF:(c + 1) * KF])
                continue

            # iter 1
            emit_softmax(c)
            pre1 = emit_preact(c)
            emit_squash(pre1[:], act_b[:, c * KF:(c + 1) * KF].rearrange(
                "p (a d) -> p a d", a=OA))
            ag = spool.tile([128, I * OD], bf16, name="ag", tag="ag")
            emit_agree(c, ag[:].rearrange("p (i d) -> p i d", i=I))
            lg = logits[:, c * I * OD:(c + 1) * I * OD]
            nc.vector.tensor_add(lg, lg, ag[:])

            # iter 2 (final)
            emit_softmax(c)
            pre2 = emit_preact(c)
            emit_squash(pre2[:], act2[:, c * KF:(c + 1) * KF].rearrange(
                "p (a d) -> p a d", a=OA))

            nc.sync.dma_start(out_d[c], act2[:, c * KF:(c + 1) * KF])


def build_program(stage=99, rep=1):
    nc = bacc.Bacc("TRN2", num_devices=NCORES, debug=False,
                   target_bir_lowering=False)
    x_d = nc.dram_tensor("x", [BL, I, A, H, W], f32, kind="ExternalInput")
    w_d = nc.dram_tensor("w", [I, A, KF], f32, kind="ExternalInput")
    bias_d = nc.dram_tensor("bias", [OD, OA], f32, kind="ExternalInput")
    out_d = nc.dram_tensor("out", [NCH, 128, KF], f32, kind="ExternalOutput")
    with tile.TileContext(nc) as tc:
        emit_kernel(tc, x_d.ap(), w_d.ap(), bias_d.ap(), out_d.ap(),
                    stage=stage, rep=rep)

    # The act-table-load pass greedily flip-flops between `exp_and_others`
    # and the Ln-capable set, paying a ~1.3us LoadActFuncSet per chunk.
    # Restrict its choices to sets containing BOTH Exp and Ln (i.e.
    # natural_log_exp_and_others, which also has Copy) by blanking the
    # other sets -- positions preserved, since act_func_set_id is an index.
    from concourse.hw_specs import get_activation_tables as _gat

    def _restricted(arch):
        AF = mybir.ActivationFunctionType
        need = {AF.Exp, AF.Ln}
        return {name: (fns if need <= fns else set())
                for name, fns in _gat(arch).items()}

    _old = bacc.get_activation_tables
    bacc.get_activation_tables = _restricted
    try:
        nc.compile()
    finally:
        bacc.get_activation_tables = _old
    return nc


_PROGRAM = None


def _get_program():
    global _PROGRAM
    if _PROGRAM is None:
        _PROGRAM = build_program()
    return _PROGRAM


def unpack_out(raw):
    """[NCH, 128, KF] (pos-chunked, features (a,d)) -> [BL, OD, OA, H, W]."""
    flat = raw.reshape(PADPOS, OA, OD)[:NPOS]          # [pos, a, d]
    flat = flat.reshape(BL, H, W, OA, OD)
    return np.ascontiguousarray(flat.transpose(0, 4, 3, 1, 2))


def kernel(x, weight, bias):
    from concourse.bass_utils import run_bass_kernel_spmd

    x = np.asarray(x, dtype=np.float32)
    weight = np.asarray(weight, dtype=np.float32)
    bias = np.asarray(bias, dtype=np.float32).reshape(OD, OA)

    nc = _get_program()
    in_maps = []
    for c in range(NCORES):
        in_maps.append({
            "x": np.ascontiguousarray(x[c * BL:(c + 1) * BL]),
            "w": np.ascontiguousarray(weight.reshape(I, A, KF)),
            "bias": bias,
        })
    res = run_bass_kernel_spmd(nc, in_maps, core_ids=list(range(NCORES)))
    out = np.empty((B, OD, OA, H, W), dtype=np.float32)
    for c in range(NCORES):
        out[c * BL:(c + 1) * BL] = unpack_out(res.results[c]["out"])
    return out

